# revision 1
# baseline (speedup 1.0000x reference)
"""Linear attention (non-causal, elu+1 feature map) on 8 Trainium2 cores.

Math per (batch b, head h), with phi(x) = elu(x)+1:
    C_aug = phi(K)^T @ [V | 1]        # (64, 65): context (64x64) + k_sum col
    numer = phi(Q) @ C_aug[:, :64]    # (T, 64)
    denom = phi(Q) @ C_aug[:, 64]     # (T,)
    out   = numer / denom             # eps=1e-6 is negligible vs denom ~1e5

Sharding: 16 heads / 8 cores = 2 heads per core, all 4 batches per core
(per-head problems are fully independent). Host pre-transposes Q per core
to (e, t) layout so every device matmul contracts along SBUF partitions
with zero on-device transposes, packs [K | V | 1] per head into one
tensor (one DMA per head -> one DMA-queue wait per matmul; the ISA allows
only 2 sync waits on a weight-load), and the ones column makes k_sum fall
out of matmul1 for free.

Device layouts (per core, all f32, all DMA-contiguous):
    qt:  (4, 128, 4096)    qt[b, hh*64+e, t] = Q[b, t, head(hh)*64+e]
    kva: (4, 2, 4096, 129) [K | V | 1] per head
    o:   (4, 2, 4096, 64)  natural per-head output

t-blocking: kva tiles assign t = p*32 + n (partition p, tile n) so each
DMA reads ~16KB contiguous per partition; matmul2 uses strided lhsT
column chunks (t = j*32 + n) so output blocks land contiguous in HBM too.
The t->(partition, tile) assignment is sum-invariant for matmul1 and
self-consistent for matmul2's output indexing.
"""

from contextlib import ExitStack

import numpy as np

import concourse.bacc as bacc
import concourse.bass as bass
import concourse.mybir as mybir
import concourse.tile as tile
from concourse.bass_utils import run_bass_kernel_spmd

B = 4
T = 4096
D = 1024
H = 16
E = 64
EA = E + 1
W = E + EA  # 129 cols per kva row
NCORES = 8
HPC = H // NCORES  # 2 heads per core
P = 128
NT = T // P  # 32 t-tiles
GRP = 4  # matmul2 chunks per psum group
DT = mybir.dt.float32
AF = mybir.ActivationFunctionType
ALU = mybir.AluOpType


def _phi(nc, x, tmp):
    """x <- elu(x)+1 == max(x+1, exp(min(x, 0))), tmp as scratch.

    x and tmp may be multi-dim APs of matching shape.
    """
    nc.vector.tensor_scalar_min(tmp, x, 0.0)
    nc.scalar.activation(tmp, tmp, AF.Exp)
    nc.vector.scalar_tensor_tensor(x, x, 1.0, tmp, ALU.add, ALU.max)


def build_nc():
    nc = bacc.Bacc("TRN2", target_bir_lowering=False, debug=False)
    qt = nc.dram_tensor("qt", [B, P, T], DT, kind="ExternalInput").ap()
    kva = nc.dram_tensor("kva", [B, HPC, T, W], DT, kind="ExternalInput").ap()
    o = nc.dram_tensor("o", [B, HPC, T, E], DT, kind="ExternalOutput").ap()

    with tile.TileContext(nc) as tc, ExitStack() as ctx:
        qt_pool = ctx.enter_context(tc.tile_pool(name="qt", bufs=2))
        mq_pool = ctx.enter_context(tc.tile_pool(name="mq", bufs=2))
        kva_pool = ctx.enter_context(tc.tile_pool(name="kva", bufs=3))
        mk_pool = ctx.enter_context(tc.tile_pool(name="mk", bufs=2))
        c_pool = ctx.enter_context(tc.tile_pool(name="c", bufs=2))
        r_pool = ctx.enter_context(tc.tile_pool(name="r", bufs=4))
        out_pool = ctx.enter_context(tc.tile_pool(name="out", bufs=2))
        psc_pool = ctx.enter_context(tc.tile_pool(name="psc", bufs=2, space="PSUM"))
        pso_pool = ctx.enter_context(tc.tile_pool(name="pso", bufs=4, space="PSUM"))

        for b in range(B):
            # Q^T for both heads: (128, 4096), partition = hh*64+e
            qt_t = qt_pool.tile([P, T], DT)
            nc.sync.dma_start(qt_t[:], qt[b])
            mq = mq_pool.tile([P, T], DT)
            _phi(nc, qt_t[:], mq[:])

            # ---- matmul1: C_aug[e, m] = sum_t phiK[t, e] * [V|1][t, m] ----
            # head 0 -> psum partitions 0:64, head 1 -> 64:128 (col tiling)
            psum_c = psc_pool.tile([P, EA], DT)
            for h in range(HPC):
                kva_t = kva_pool.tile([P, NT * W], DT)
                nc.sync.dma_start(
                    kva_t[:].rearrange("p (n e) -> p n e", e=W),
                    kva[b, h].rearrange("(p n) e -> p n e", p=P),
                )
                k3 = kva_t[:].rearrange("p (n e) -> p n e", e=W)[:, :, 0:E]
                mk = mk_pool.tile([P, NT * E], DT)
                _phi(nc, k3, mk[:].rearrange("p (n e) -> p n e", e=E))
                for n in range(NT):
                    nc.tensor.matmul(
                        psum_c[h * E : (h + 1) * E, :],
                        lhsT=kva_t[:, n * W : n * W + E],
                        rhs=kva_t[:, n * W + E : (n + 1) * W],
                        start=(n == 0),
                        stop=(n == NT - 1),
                        tile_position=(0, h * E),
                    )
            c_sb = c_pool.tile([P, EA], DT)
            nc.vector.tensor_copy(c_sb[:], psum_c[:])

            # ---- matmul2 + normalize: out[t, d] = phiQ[t,:] @ C[:, d] / denom[t]
            for h in range(HPC):
                out_sb = out_pool.tile([P, NT * E], DT)
                for g in range(NT // GRP):
                    ps_o = pso_pool.tile([P, GRP * EA], DT)
                    for j in range(GRP):
                        n = g * GRP + j
                        nc.tensor.matmul(
                            ps_o[:, j * EA : (j + 1) * EA],
                            lhsT=qt_t[h * E : (h + 1) * E, n::NT],
                            rhs=c_sb[h * E : (h + 1) * E, :],
                            start=True,
                            stop=True,
                        )
                    r_sb = r_pool.tile([P, GRP], DT)
                    nc.vector.reciprocal(r_sb[:], ps_o[:, E::EA])
                    for j in range(GRP):
                        n = g * GRP + j
                        nc.vector.tensor_scalar_mul(
                            out_sb[:, n * E : (n + 1) * E],
                            ps_o[:, j * EA : j * EA + E],
                            r_sb[:, j : j + 1],
                        )
                nc.sync.dma_start(
                    o[b, h].rearrange("(p n) e -> p n e", p=P),
                    out_sb[:].rearrange("p (n e) -> p n e", e=E),
                )
    nc.finalize()
    return nc


_NC_CACHE = None


def _get_nc():
    global _NC_CACHE
    if _NC_CACHE is None:
        _NC_CACHE = build_nc()
    return _NC_CACHE


def make_in_maps(query, key, value):
    query = np.ascontiguousarray(query, dtype=np.float32)
    key = np.ascontiguousarray(key, dtype=np.float32)
    value = np.ascontiguousarray(value, dtype=np.float32)
    in_maps = []
    for c in range(NCORES):
        lo = c * HPC * E
        hi = lo + HPC * E
        qt = np.ascontiguousarray(query[:, :, lo:hi].transpose(0, 2, 1))
        kva = np.empty((B, HPC, T, W), np.float32)
        kva[..., :E] = key[:, :, lo:hi].reshape(B, T, HPC, E).transpose(0, 2, 1, 3)
        kva[..., E : E + E] = (
            value[:, :, lo:hi].reshape(B, T, HPC, E).transpose(0, 2, 1, 3)
        )
        kva[..., E + E] = 1.0
        in_maps.append({"qt": qt, "kva": kva})
    return in_maps


def assemble_out(results):
    out = np.empty((B, T, D), np.float32)
    for c in range(NCORES):
        oc = results[c]["o"]  # (B, HPC, T, E)
        out[:, :, c * HPC * E : (c + 1) * HPC * E] = oc.transpose(0, 2, 1, 3).reshape(
            B, T, HPC * E
        )
    return out


def run(query, key, value, **spmd_kwargs):
    nc = _get_nc()
    in_maps = make_in_maps(query, key, value)
    res = run_bass_kernel_spmd(nc, in_maps, core_ids=list(range(NCORES)), **spmd_kwargs)
    return assemble_out(res.results), res


def kernel(query, key, value):
    out, _ = run(query, key, value)
    return out



# revision 6
# speedup vs baseline: 2.0696x; 2.0696x over previous
"""Linear attention (non-causal, elu+1 feature map) on 8 Trainium2 cores.

Math per (batch b, head h), with phi(x) = elu(x)+1 = max(x+1, exp(min(x,0))):
    C_aug = phi(K)^T @ [V | 1]        # (64, 65): context (64x64) + k_sum col
    numer = phi(Q) @ C_aug[:, :64]    # (T, 64)
    denom = phi(Q) @ C_aug[:, 64]     # (T,)
    out   = numer / denom             # eps=1e-6 negligible vs denom ~1e5

Sharding: 16 heads / 8 cores = 2 heads per core, all 4 batches per core.

Everything on device is fp16 (inputs converted on host, output upcast on
host): halves HBM traffic vs fp32 and runs the PE at 4x the fp32 rate.
fp16 (not bf16): the 10-bit mantissa keeps worst-element rel err ~1.3e-2
vs the 2e-2 budget (bf16 would land ~9.6e-2); all values fit fp16 range
(|C|<=500, ksum<=5000, |out|<=0.07).

Both heads are fused into single 128-wide matmuls:
  mm1: lhsT = [phiK0 | phiK1] (128t x 128), rhs = [VA0 | VA1] (128t x 130)
       -> psum (128 x 130); the two diagonal 64x65 blocks are C_aug per
       head (cross-head blocks are discarded), accumulated over 32 t-tiles.
  mm2: lhsT = phiQ chunk (128e x 128t), rhs = block-diag C (128 x 130)
       -> psum (128t x 130) = [numer0|denom0|numer1|denom1] per t-chunk.
Halves instruction count and makes every weight load a full 128 columns.

Device layouts (per core, all fp16, partition dim first, all APs dense):
    qt: (B, 128, 4096)  qt[b, hh*64+e, n*128+j] = Q[b, t=j*32+n, ch]
    kv: (B, 128, 8256)  cols 0:4096   = K  [n, h, e] (n*128+h*64+e)
                        cols 4096:8256= VA [n, h, m] (n*130+h*65+m, m=64 ones)
                        partition p <-> t = p*32+n
    o:  (B, 128, 4096)  o[b, p, n*128+h*64+e] = out[b, t=p*32+n, h*64+e]
The t = p*32+n tiling gives every DMA 4-8 KB contiguous per partition.
"""

from contextlib import ExitStack

import numpy as np
import ml_dtypes

import concourse.bacc as bacc
import concourse.bass as bass
import concourse.mybir as mybir
import concourse.tile as tile
from concourse.bass_utils import run_bass_kernel_spmd

B = 4
T = 4096
D = 1024
H = 16
E = 64
EA = E + 1
W2 = 2 * EA  # 130 cols: both heads' [C | ksum]
NCORES = 8
HPC = H // NCORES  # 2 heads per core
P = 128
NT = T // P  # 32 t-tiles
KC = HPC * NT * E  # 4096 k-region cols
VC = HPC * NT * EA  # 4160 va-region cols
KV = KC + VC  # 8256
GRP = 2  # mm2 chunks per psum bank (2*130 fp32 = 1040 B <= 2 KB)
DT = mybir.dt.float16
F32 = mybir.dt.float32
AF = mybir.ActivationFunctionType
ALU = mybir.AluOpType
F16 = np.float16


def _phi(nc, x, tmp):
    """x <- elu(x)+1 == max(x+1, exp(min(x, 0))), tmp as scratch."""
    nc.vector.tensor_scalar_min(tmp, x, 0.0)
    nc.scalar.activation(tmp, tmp, AF.Exp)
    nc.vector.scalar_tensor_tensor(x, x, 1.0, tmp, ALU.add, ALU.max)


def build_nc():
    nc = bacc.Bacc("TRN2", target_bir_lowering=False, debug=False)
    qt = nc.dram_tensor("qt", [B, P, T], DT, kind="ExternalInput").ap()
    kv = nc.dram_tensor("kv", [B, P, KV], DT, kind="ExternalInput").ap()
    o = nc.dram_tensor("o", [B, P, T], DT, kind="ExternalOutput").ap()

    with tile.TileContext(nc) as tc, ExitStack() as ctx:
        qt_pool = ctx.enter_context(tc.tile_pool(name="qt", bufs=2))
        kv_pool = ctx.enter_context(tc.tile_pool(name="kv", bufs=2))
        tmp_pool = ctx.enter_context(tc.tile_pool(name="tmp", bufs=2))
        c_pool = ctx.enter_context(tc.tile_pool(name="c", bufs=2))
        r_pool = ctx.enter_context(tc.tile_pool(name="r", bufs=4))
        out_pool = ctx.enter_context(tc.tile_pool(name="out", bufs=2))
        psc_pool = ctx.enter_context(tc.tile_pool(name="psc", bufs=2, space="PSUM"))
        pso_pool = ctx.enter_context(tc.tile_pool(name="pso", bufs=4, space="PSUM"))

        for b in range(B):
            qt_t = qt_pool.tile([P, T], DT)
            nc.sync.dma_start(qt_t[:], qt[b])
            kv_t = kv_pool.tile([P, KV], DT)
            nc.sync.dma_start(kv_t[:], kv[b])

            tq = tmp_pool.tile([P, T], DT)
            _phi(nc, qt_t[:], tq[:])
            tk = tmp_pool.tile([P, KC], DT)
            _phi(nc, kv_t[:, 0:KC], tk[:])

            # ---- mm1: psum[(hi,e),(hj,m)] = sum_t phiK_hi[t,e]*VA_hj[t,m] ----
            # k cols are (n, h, e): tile n = cols n*128..(n+1)*128
            # va cols are (n, h, m): tile n = cols KC+n*130..KC+(n+1)*130
            psc = psc_pool.tile([P, W2], F32)
            for n in range(NT):
                nc.tensor.matmul(
                    psc[:],
                    lhsT=kv_t[:, n * P : (n + 1) * P],
                    rhs=kv_t[:, KC + n * W2 : KC + (n + 1) * W2],
                    start=(n == 0),
                    stop=(n == NT - 1),
                )
            # block-diag C in bf16: diag blocks of psc, zeros elsewhere
            c_sb = c_pool.tile([P, W2], DT)
            nc.vector.memset(c_sb[:], 0.0)
            nc.vector.tensor_copy(c_sb[0:E, 0:EA], psc[0:E, 0:EA])
            nc.vector.tensor_copy(c_sb[E:P, EA:W2], psc[E:P, EA:W2])

            # ---- mm2 + normalize ----
            out_sb = out_pool.tile([P, T], DT)
            for g in range(NT // GRP):
                pso = pso_pool.tile([P, GRP * W2], F32)
                for j in range(GRP):
                    n = g * GRP + j
                    nc.tensor.matmul(
                        pso[:, j * W2 : (j + 1) * W2],
                        lhsT=qt_t[:, n * P : (n + 1) * P],
                        rhs=c_sb[:],
                        start=True,
                        stop=True,
                    )
                r_sb = r_pool.tile([P, GRP * HPC], F32)
                nc.vector.reciprocal(r_sb[:], pso[:, E::EA])
                ov = out_sb[:, g * GRP * P : (g + 1) * GRP * P].rearrange(
                    "p (n h e) -> p n h e", n=GRP, h=HPC
                )
                iv = pso[:].rearrange("p (n h x) -> p n h x", n=GRP, h=HPC)[
                    :, :, :, 0:E
                ]
                rv = (
                    r_sb[:]
                    .rearrange("p (n h) -> p n h", h=HPC)
                    .unsqueeze(3)
                    .broadcast_to((P, GRP, HPC, E))
                )
                nc.vector.tensor_tensor(ov, iv, rv, ALU.mult)
            nc.sync.dma_start(o[b], out_sb[:])
    nc.finalize()
    return nc


_NC_CACHE = None


def _get_nc():
    global _NC_CACHE
    if _NC_CACHE is None:
        _NC_CACHE = build_nc()
    return _NC_CACHE


def make_in_maps(query, key, value):
    query = np.ascontiguousarray(query, dtype=np.float32)
    key = np.ascontiguousarray(key, dtype=np.float32)
    value = np.ascontiguousarray(value, dtype=np.float32)
    in_maps = []
    for c in range(NCORES):
        lo = c * P
        hi = lo + P
        # qt: col n*128+j <-> t = j*32+n
        qt = query[:, :, lo:hi].transpose(0, 2, 1)  # (B, 128, T) t-major
        qt = np.ascontiguousarray(
            qt.reshape(B, P, P, NT).transpose(0, 1, 3, 2)
        ).reshape(B, P, T)
        # k region: (B, p, n, h, e); t = p*32+n
        kk = key[:, :, lo:hi].reshape(B, P, NT, HPC, E)
        # va region: ones col appended per head, cols (n, h, m)
        va = np.empty((B, P, NT, HPC, EA), np.float32)
        va[..., :E] = value[:, :, lo:hi].reshape(B, P, NT, HPC, E)
        va[..., E] = 1.0
        kvb = np.concatenate(
            [kk.reshape(B, P, KC), va.reshape(B, P, VC)], axis=2
        )
        in_maps.append(
            {"qt": qt.astype(F16), "kv": np.ascontiguousarray(kvb).astype(F16)}
        )
    return in_maps


def assemble_out(results):
    out = np.empty((B, T, D), np.float32)
    for c in range(NCORES):
        oc = np.asarray(results[c]["o"], dtype=np.float32)  # (B, 128, 4096)
        # col = n*128 + h*64 + e; partition p <-> t = p*32+n
        out[:, :, c * P : (c + 1) * P] = oc.reshape(B, T, P)
    return out


def run(query, key, value, **spmd_kwargs):
    nc = _get_nc()
    in_maps = make_in_maps(query, key, value)
    res = run_bass_kernel_spmd(nc, in_maps, core_ids=list(range(NCORES)), **spmd_kwargs)
    return assemble_out(res.results), res


def kernel(query, key, value):
    out, _ = run(query, key, value)
    return out


# revision 9
# speedup vs baseline: 2.5828x; 1.2480x over previous
"""Linear attention (non-causal, elu+1 feature map) on 8 Trainium2 cores.

Math per (batch b, head h), with phi(x) = elu(x)+1 = max(x+1, exp(min(x,0))):
    C_aug = phi(K)^T @ [V | 1]        # (64, 65): context (64x64) + k_sum col
    numer = phi(Q) @ C                # (T, 64)
    denom = phi(Q) @ k_sum            # (T,)
    out   = numer / denom             # eps=1e-6 negligible vs denom ~1e5

Sharding: 16 heads / 8 cores = 2 heads per core, all 4 batches per core.

Everything on device is fp16: halves HBM traffic vs fp32 and the PE runs
fp16 at the same per-column rate as bf16 (measured), while fp16's 10-bit
mantissa keeps worst-element rel err ~1.3e-2 vs the 2e-2 budget (bf16
lands ~9.6e-2). All values fit fp16 range (|C|<=500, ksum<=5000).

Both heads are fused into single 128-wide matmuls:
  mm1: lhsT = [phiK0 | phiK1] (128t x 128), rhs = [VA0 | VA1] (128t x 130)
       -> psum (128 x 130); diagonal 64x65 blocks are C_aug per head
       (cross-head blocks discarded), accumulated over 32 t-tiles.
  mm_d: lhsT = phiQ chunk (128e x 128t), rhs = blockdiag ksum (128 x 2)
       -> denom psum (128t x 2) per chunk, all 32 chunks in one bank so a
       single reciprocal per batch covers them (recip is slow per-call).
  mm2: same lhsT, rhs = blockdiag C (128 x 128) -> numer psum (128t x 128);
       4 chunks fill one 2 KB psum bank exactly, so the normalize+evac is
       one dense 512-col DVE multiply per group (psum fp32 forces 1x mode;
       density and batching are all that's left to win).

phi pipelining: phi(K) runs in 2 column-chunks so mm1 tiles 0..15 start
as soon as the first half is done; phi(Q) overlaps mm1 on the DVE.
phi is 3 ops: tensor_scalar min (4x mode), ACT exp, tensor_scalar add +
tensor max (the fused scalar_tensor_tensor measured 1x; add runs 4x).

Device layouts (per core, all fp16, partition dim first, all APs dense):
    qt: (B, 128, 4096)  qt[b, hh*64+e, n*128+j] = Q[b, t=j*32+n, ch]
    kv: (B, 128, 8256)  cols 0:4096   = K  [n, h, e] (n*128+h*64+e)
                        cols 4096:8256= VA [n, h, m] (n*130+h*65+m, m=64 ones)
                        partition p <-> t = p*32+n
    o:  (B, 128, 4096)  o[b, p, n*128+h*64+e] = out[b, t=p*32+n, h*64+e]
The t = p*32+n tiling gives every DMA 4-8 KB contiguous per partition.
"""

from contextlib import ExitStack

import numpy as np

import concourse.bacc as bacc
import concourse.bass as bass
import concourse.mybir as mybir
import concourse.tile as tile
from concourse.bass_utils import run_bass_kernel_spmd

B = 4
T = 4096
D = 1024
H = 16
E = 64
EA = E + 1
W2 = 2 * EA  # 130 cols: both heads' [VA]
NCORES = 8
HPC = H // NCORES  # 2 heads per core
P = 128
NT = T // P  # 32 t-tiles
KC = HPC * NT * E  # 4096 k-region cols
VC = HPC * NT * EA  # 4160 va-region cols
KV = KC + VC  # 8256
GRP = 4  # mm2 chunks per psum bank (4*128 fp32 = 2048 B = full bank)
NKCH = 2  # phi(K) column chunks
DT = mybir.dt.float16
F32 = mybir.dt.float32
AF = mybir.ActivationFunctionType
ALU = mybir.AluOpType
F16 = np.float16


def _phi(nc, x, tmp):
    """x <- elu(x)+1 == max(x+1, exp(min(x, 0))), tmp as scratch."""
    nc.vector.tensor_scalar_min(tmp, x, 0.0)
    nc.scalar.activation(tmp, tmp, AF.Exp)
    nc.vector.tensor_scalar_add(x, x, 1.0)
    nc.vector.tensor_tensor(x, x, tmp, ALU.max)


def build_nc():
    nc = bacc.Bacc("TRN2", target_bir_lowering=False, debug=False)
    qt = nc.dram_tensor("qt", [B, P, T], DT, kind="ExternalInput").ap()
    kv = nc.dram_tensor("kv", [B, P, KV], DT, kind="ExternalInput").ap()
    o = nc.dram_tensor("o", [B, P, T], DT, kind="ExternalOutput").ap()

    with tile.TileContext(nc) as tc, ExitStack() as ctx:
        qt_pool = ctx.enter_context(tc.tile_pool(name="qt", bufs=2))
        kv_pool = ctx.enter_context(tc.tile_pool(name="kv", bufs=2))
        tmpk_pool = ctx.enter_context(tc.tile_pool(name="tmpk", bufs=2))
        tmpq_pool = ctx.enter_context(tc.tile_pool(name="tmpq", bufs=2))
        c_pool = ctx.enter_context(tc.tile_pool(name="c", bufs=2))
        ks_pool = ctx.enter_context(tc.tile_pool(name="ks", bufs=2))
        r_pool = ctx.enter_context(tc.tile_pool(name="r", bufs=2))
        out_pool = ctx.enter_context(tc.tile_pool(name="out", bufs=2))
        psc_pool = ctx.enter_context(tc.tile_pool(name="psc", bufs=2, space="PSUM"))
        pso_pool = ctx.enter_context(tc.tile_pool(name="pso", bufs=4, space="PSUM"))
        psd_pool = ctx.enter_context(tc.tile_pool(name="psd", bufs=2, space="PSUM"))

        for b in range(B):
            kv_t = kv_pool.tile([P, KV], DT)
            nc.sync.dma_start(kv_t[:], kv[b])
            qt_t = qt_pool.tile([P, T], DT)
            nc.sync.dma_start(qt_t[:], qt[b])

            # phi(K) in chunks; mm1 tiles chase each chunk
            psc = psc_pool.tile([P, W2], F32)
            kchunk = KC // NKCH
            tpc = NT // NKCH  # t-tiles per chunk
            for c in range(NKCH):
                tk = tmpk_pool.tile([P, kchunk], DT)
                _phi(nc, kv_t[:, c * kchunk : (c + 1) * kchunk], tk[:])
                for n in range(c * tpc, (c + 1) * tpc):
                    nc.tensor.matmul(
                        psc[:],
                        lhsT=kv_t[:, n * P : (n + 1) * P],
                        rhs=kv_t[:, KC + n * W2 : KC + (n + 1) * W2],
                        start=(n == 0),
                        stop=(n == NT - 1),
                    )
            # phi(Q): overlaps mm1 on the vector engine
            tq = tmpq_pool.tile([P, T], DT)
            _phi(nc, qt_t[:], tq[:])

            # block-diag C (numer cols only) and ksum, in fp16
            c_sb = c_pool.tile([P, P], DT)
            nc.vector.memset(c_sb[:], 0.0)
            nc.vector.tensor_copy(c_sb[0:E, 0:E], psc[0:E, 0:E])
            nc.vector.tensor_copy(c_sb[E:P, E:P], psc[E:P, EA : EA + E])
            ks_sb = ks_pool.tile([P, HPC], DT)
            nc.vector.memset(ks_sb[:], 0.0)
            nc.vector.tensor_copy(ks_sb[0:E, 0:1], psc[0:E, E : E + 1])
            nc.vector.tensor_copy(ks_sb[E:P, 1:2], psc[E:P, EA + E : W2])

            # denominators for all 32 chunks into one psum bank
            psd = psd_pool.tile([P, NT * HPC], F32)
            for n in range(NT):
                nc.tensor.matmul(
                    psd[:, n * HPC : (n + 1) * HPC],
                    lhsT=qt_t[:, n * P : (n + 1) * P],
                    rhs=ks_sb[:],
                    start=True,
                    stop=True,
                )
            r_sb = r_pool.tile([P, NT * HPC], F32)
            nc.vector.reciprocal(r_sb[:], psd[:])

            # numerators + normalize, 4 chunks per psum bank
            out_sb = out_pool.tile([P, T], DT)
            for g in range(NT // GRP):
                pso = pso_pool.tile([P, GRP * P], F32)
                for j in range(GRP):
                    n = g * GRP + j
                    nc.tensor.matmul(
                        pso[:, j * P : (j + 1) * P],
                        lhsT=qt_t[:, n * P : (n + 1) * P],
                        rhs=c_sb[:],
                        start=True,
                        stop=True,
                    )
                ov = out_sb[:, g * GRP * P : (g + 1) * GRP * P].rearrange(
                    "p (n h e) -> p n h e", n=GRP, h=HPC
                )
                iv = pso[:].rearrange("p (n h e) -> p n h e", n=GRP, h=HPC)
                rv = (
                    r_sb[:, g * GRP * HPC : (g + 1) * GRP * HPC]
                    .rearrange("p (n h) -> p n h", h=HPC)
                    .unsqueeze(3)
                    .broadcast_to((P, GRP, HPC, E))
                )
                nc.vector.tensor_tensor(ov, iv, rv, ALU.mult)
            nc.sync.dma_start(o[b], out_sb[:])
    nc.finalize()
    return nc


_NC_CACHE = None


def _get_nc():
    global _NC_CACHE
    if _NC_CACHE is None:
        _NC_CACHE = build_nc()
    return _NC_CACHE


def make_in_maps(query, key, value):
    query = np.ascontiguousarray(query, dtype=np.float32)
    key = np.ascontiguousarray(key, dtype=np.float32)
    value = np.ascontiguousarray(value, dtype=np.float32)
    in_maps = []
    for c in range(NCORES):
        lo = c * P
        hi = lo + P
        # qt: col n*128+j <-> t = j*32+n
        qt = query[:, :, lo:hi].transpose(0, 2, 1)  # (B, 128, T) t-major
        qt = np.ascontiguousarray(
            qt.reshape(B, P, P, NT).transpose(0, 1, 3, 2)
        ).reshape(B, P, T)
        # k region: (B, p, n, h, e); t = p*32+n
        kk = key[:, :, lo:hi].reshape(B, P, NT, HPC, E)
        # va region: ones col appended per head, cols (n, h, m)
        va = np.empty((B, P, NT, HPC, EA), np.float32)
        va[..., :E] = value[:, :, lo:hi].reshape(B, P, NT, HPC, E)
        va[..., E] = 1.0
        kvb = np.concatenate(
            [kk.reshape(B, P, KC), va.reshape(B, P, VC)], axis=2
        )
        in_maps.append(
            {"qt": qt.astype(F16), "kv": np.ascontiguousarray(kvb).astype(F16)}
        )
    return in_maps


def assemble_out(results):
    out = np.empty((B, T, D), np.float32)
    for c in range(NCORES):
        oc = np.asarray(results[c]["o"], dtype=np.float32)  # (B, 128, 4096)
        # col = n*128 + h*64 + e; partition p <-> t = p*32+n
        out[:, :, c * P : (c + 1) * P] = oc.reshape(B, T, P)
    return out


def run(query, key, value, **spmd_kwargs):
    nc = _get_nc()
    in_maps = make_in_maps(query, key, value)
    res = run_bass_kernel_spmd(nc, in_maps, core_ids=list(range(NCORES)), **spmd_kwargs)
    return assemble_out(res.results), res


def kernel(query, key, value):
    out, _ = run(query, key, value)
    return out
